# revision 1
# baseline (speedup 1.0000x reference)
"""Trainium2 Bass kernel for nn_NodePreTrans (e3nn tensor product + linear).

Data-parallel over nodes: 50000 rows sharded 8 ways (6250/core, padded to
6272).  Channel-major on-device layout: all matmuls are (weights stationary)
[K,128] x [K,Z] with Z up to 512 nodes in the moving/free dimension.
"""

import sys

sys.path.insert(0, "/opt/trn_rl_repo")

import numpy as np

import concourse.bacc as bacc
import concourse.bass as bass
import concourse.mybir as mybir
import concourse.tile as tile
from concourse.bass_utils import run_bass_kernel_spmd

N_NODES = 50000
N_CORES = 8
NS = N_NODES // N_CORES          # 6250 real nodes per core
NSH = 6272                       # padded (12*512 + 128)
MUL_S = 256
MUL_V = 128

C_000 = 1.0 / np.sqrt(256.0)
C_011 = 1.0 / np.sqrt(128.0)
C_101 = 1.0 / np.sqrt(256.0)
C_110 = 1.0 / np.sqrt(384.0)
C_111 = 1.0 / 16.0

F32 = mybir.dt.float32
F32R = mybir.dt.float32r
BF16 = mybir.dt.bfloat16

_CACHE = {}

VARIANT = "full"


def _build_program(variant="full"):
    nc = bacc.Bacc("TRN2", target_bir_lowering=False, debug=False,
                   num_devices=N_CORES)

    if variant == "b16":
        MDT = BF16
    elif variant in ("dma", "mm"):
        MDT = F32
    else:
        MDT = F32R
    ODT = BF16 if variant == "b16" else F32
    xT_d = nc.dram_tensor("xT", [640, NSH], MDT, kind="ExternalInput").ap()
    wt000_d = nc.dram_tensor("wt000", [256, 256], MDT, kind="ExternalInput").ap()
    wt011_d = nc.dram_tensor("wt011", [128, 256], MDT, kind="ExternalInput").ap()
    wt101_d = nc.dram_tensor("wt101", [256, 128], MDT, kind="ExternalInput").ap()
    wt110_d = nc.dram_tensor("wt110", [128, 128], MDT, kind="ExternalInput").ap()
    wt111_d = nc.dram_tensor("wt111", [128, 128], MDT, kind="ExternalInput").ap()
    l0e_d = nc.dram_tensor("l0e", [384, 256], MDT, kind="ExternalInput").ap()
    l1o_d = nc.dram_tensor("l1o", [384, 128], MDT, kind="ExternalInput").ap()
    l1e_d = nc.dram_tensor("l1e", [128, 128], MDT, kind="ExternalInput").ap()
    outT_d = nc.dram_tensor("outT", [1024, NSH], ODT, kind="ExternalOutput").ap()

    with tile.TileContext(nc) as tc:
        _emit(tc, nc, xT_d, wt000_d, wt011_d, wt101_d, wt110_d, wt111_d,
              l0e_d, l1o_d, l1e_d, outT_d, variant=variant, mdt=MDT)

    nc.compile()
    return nc


def _emit(tc, nc, xT_d, wt000_d, wt011_d, wt101_d, wt110_d, wt111_d,
          l0e_d, l1o_d, l1e_d, outT_d, variant="full", mdt=F32R):
    if variant == "b16":
        TW = 1024                      # bf16 moving + bf16 PSUM bank limit
        PDT = EDT = BF16
        zblocks = [(i * 1024, 1024) for i in range(6)] + [(6144, 128)]
    else:
        TW = 512                       # f32 PSUM bank limit
        PDT = EDT = F32
        zblocks = [(i * 512, 512) for i in range(12)] + [(6144, 128)]
    with (
        tc.tile_pool(name="wpool", bufs=1) as wpool,
        tc.tile_pool(name="xin", bufs=3) as xin,
        tc.tile_pool(name="gat", bufs=2) as gat,
        tc.tile_pool(name="tmp", bufs=4) as tmp,
        tc.tile_pool(name="oev", bufs=2) as oev,
        tc.tile_pool(name="ps1", bufs=1, space="PSUM") as ps1,
        tc.tile_pool(name="ps2", bufs=1, space="PSUM") as ps2,
    ):
        # ---- resident weights -------------------------------------------
        # trigger weight loads from ACT so they don't queue ahead of the
        # x-tile loads on Sync's DGE ring; order them by first use.
        def wtile(name, dram_ap, rows, cols):
            t = wpool.tile([128, cols], mdt, name=name)
            nc.scalar.dma_start(t[:, :], dram_ap[rows:rows + 128, :])
            return t

        w111 = wtile("w111", wt111_d, 0, 128)
        w110 = wtile("w110", wt110_d, 0, 128)
        w011 = wtile("w011", wt011_d, 0, 256)
        w000 = [wtile(f"w000_{k}", wt000_d, 128 * k, 256) for k in range(2)]
        w101 = [wtile(f"w101_{k}", wt101_d, 128 * k, 128) for k in range(2)]
        L1e = wtile("l1e", l1e_d, 0, 128)
        L1o = [wtile(f"l1o_{k}", l1o_d, 128 * k, 128) for k in range(3)]
        L0e = [wtile(f"l0e_{k}", l0e_d, 128 * k, 256) for k in range(3)]

        for bi, (z0, Z) in enumerate(zblocks):
            # ---- load x tiles (channel-major); v first (path 5 needs it)
            def load(t, row0, Z=Z, z0=z0):
                nc.sync.dma_start(t[:, :Z], xT_d[row0:row0 + 128,
                                                 z0:z0 + Z])

            v = []
            for j in range(3):
                t = xin.tile([128, TW], mdt, name=f"v{j}")
                load(t, 256 + 128 * j)
                v.append(t)
            s = []
            for m in range(2):
                t = xin.tile([128, TW], mdt, name=f"s{m}")
                load(t, 128 * m)
                s.append(t)

            def ps_tile():
                return ps1.tile([128, TW], PDT, name="s1r", bufs=5)

            def mmr(out, lhsT, rhs, start, stop):
                nc.tensor.matmul(out, lhsT, rhs, start=start, stop=stop)

            if variant == "dma":
                for i, t in enumerate(s + v):
                    nc.sync.dma_start(outT_d[128 * i:128 * (i + 1),
                                             z0:z0 + Z], t[:, :Z])
                continue

            if variant == "mm":
                idx = 0
                for (w, rr) in [(w000[0], s[0]), (w000[1], s[1]),
                                (w011, v[0]), (w011, v[1]), (w011, v[2]),
                                (w101[0], s[0]), (w101[1], s[1]),
                                (w110, v[0]), (w110, v[1]), (w110, v[2]),
                                (w111, v[0]), (w111, v[1]), (w111, v[2])]:
                    a = ps_tile()
                    nc.tensor.matmul(a[:, :Z], w[:, :128], rr[:, :Z],
                                     start=True, stop=True)
                    ev = oev.tile([128, TW], EDT, name=f"mmev{idx % 4}")
                    nc.scalar.copy(ev[:, :Z], a[:, :Z])
                    nc.sync.dma_start(outT_d[128 * (idx % 8):
                                             128 * (idx % 8) + 128,
                                             z0:z0 + Z], ev[:, :Z])
                    idx += 1
                continue

            # ---- path 5: p5_k = v_i*E_j - v_j*E_i, (i,j)=(k+1,k+2)%3 ---
            # muls on DVE (read E from PSUM); final subs on GpSimd (SBUF-only)
            E = []
            for j in range(3):
                e = ps_tile()
                mmr(e[:, :Z], w111[:, :], v[j][:, :Z], start=True, stop=True)
                E.append(e)
            p5 = []
            for k in range(3):
                i, j = (k + 1) % 3, (k + 2) % 3
                ta = tmp.tile([128, TW], mdt, name="t5a")
                tb = tmp.tile([128, TW], mdt, name="t5b")
                nc.vector.tensor_mul(ta[:, :Z], v[i][:, :Z], E[j][:, :Z])
                nc.vector.tensor_mul(tb[:, :Z], v[j][:, :Z], E[i][:, :Z])
                p = gat.tile([128, TW], mdt, name=f"p5_{k}")
                nc.gpsimd.tensor_sub(p[:, :Z], ta[:, :Z], tb[:, :Z])
                p5.append(p)

            # ---- path 1: p1 = s * (w00.T @ s) --------------------------
            p1 = []
            for m in range(2):
                a = ps_tile()
                mmr(a[:, :Z], w000[0][:, 128 * m:128 * (m + 1)],
                    s[0][:, :Z], start=True, stop=False)
                mmr(a[:, :Z], w000[1][:, 128 * m:128 * (m + 1)],
                    s[1][:, :Z], start=False, stop=True)
                p = gat.tile([128, TW], mdt, name=f"p1_{m}")
                nc.vector.tensor_mul(p[:, :Z], s[m][:, :Z], a[:, :Z])
                p1.append(p)

            # ---- path 2: p2_j = s * (w01.T @ v_j) ----------------------
            p2 = []
            for j in range(3):
                pj = []
                for m in range(2):
                    b = ps_tile()
                    mmr(b[:, :Z], w011[:, 128 * m:128 * (m + 1)],
                        v[j][:, :Z], start=True, stop=True)
                    p = gat.tile([128, TW], mdt, name=f"p2_{j}_{m}")
                    nc.vector.tensor_mul(p[:, :Z], s[m][:, :Z], b[:, :Z])
                    pj.append(p)
                p2.append(pj)

            # ---- path 3: p3_j = v_j * (w10.T @ s) ----------------------
            c = ps_tile()
            mmr(c[:, :Z], w101[0][:, :], s[0][:, :Z], start=True, stop=False)
            mmr(c[:, :Z], w101[1][:, :], s[1][:, :Z], start=False, stop=True)
            p3 = []
            for j in range(3):
                p = gat.tile([128, TW], mdt, name=f"p3_{j}")
                nc.vector.tensor_mul(p[:, :Z], v[j][:, :Z], c[:, :Z])
                p3.append(p)

            # ---- path 4: p4 = sum_j v_j * (w110.T @ v_j) ---------------
            # muls on DVE (read PSUM); accumulate adds on GpSimd (SBUF-only)
            p4 = gat.tile([128, TW], mdt, name="p4")
            for j in range(3):
                d = ps_tile()
                mmr(d[:, :Z], w110[:, :], v[j][:, :Z], start=True, stop=True)
                if j == 0:
                    nc.vector.tensor_mul(p4[:, :Z], v[0][:, :Z], d[:, :Z])
                else:
                    t4 = tmp.tile([128, TW], mdt, name="t4")
                    nc.vector.tensor_mul(t4[:, :Z], v[j][:, :Z], d[:, :Z])
                    nc.gpsimd.tensor_add(p4[:, :Z], p4[:, :Z], t4[:, :Z])

            if variant == "gat":
                outs8 = [p1[0], p1[1], p2[0][0], p2[0][1], p3[0], p4,
                         p5[0], p5[1]]
                for i, t in enumerate(outs8):
                    nc.sync.dma_start(outT_d[128 * i:128 * (i + 1),
                                             z0:z0 + Z], t[:, :Z])
                continue

            # ---- stage 2 linears + evacuate + store --------------------
            # last two blocks: split store triggers across Sync+ACT so the
            # end-of-kernel triggers don't serialize (ACT is idle by then)
            tail = bi >= len(zblocks) - 2
            oidx = [0]

            def emit_out(name, row0, chunks):
                o = ps2.tile([128, TW], PDT, name="s2o", bufs=3)
                n = len(chunks)
                for ci, (lw, rhs) in enumerate(chunks):
                    mmr(o[:, :Z], lw, rhs[:, :Z],
                        start=(ci == 0), stop=(ci == n - 1))
                ev = oev.tile([128, TW], EDT, name=name)
                nc.scalar.copy(ev[:, :Z], o[:, :Z])
                eng = nc.scalar if (tail and oidx[0] % 2) else nc.sync
                oidx[0] += 1
                eng.dma_start(outT_d[row0:row0 + 128, z0:z0 + Z],
                              ev[:, :Z])

            for j in range(3):
                emit_out(f"o1e_{j}", 640 + 128 * j, [(L1e[:, :], p5[j])])
            for j in range(3):
                tp1o = [p2[j][0], p2[j][1], p3[j]]
                emit_out(f"o1o_{j}", 256 + 128 * j,
                         [(L1o[ci][:, :], tp1o[ci]) for ci in range(3)])
            tp0e = [p1[0], p1[1], p4]
            for m in range(2):
                emit_out(f"o0e_{m}", 128 * m,
                         [(L0e[ci][:, 128 * m:128 * (m + 1)], tp0e[ci])
                          for ci in range(3)])


def _prep_inputs(node_feat, w_00_0, w_01_1, w_10_1, w_11_0, w_11_1,
                 W_0e, W_1o, W_1e, b16=False):
    ndt = np.float32
    if b16:
        import ml_dtypes
        ndt = ml_dtypes.bfloat16
    weights = {
        "wt000": np.ascontiguousarray((C_000 * w_00_0).T).astype(ndt),
        "wt011": np.ascontiguousarray((C_011 * w_01_1).T).astype(ndt),
        "wt101": np.ascontiguousarray((C_101 * w_10_1).T).astype(ndt),
        "wt110": np.ascontiguousarray((C_110 * w_11_0).T).astype(ndt),
        "wt111": np.ascontiguousarray((C_111 * w_11_1).T).astype(ndt),
        "l0e": np.ascontiguousarray(W_0e / np.sqrt(384.0)).astype(ndt),
        "l1o": np.ascontiguousarray(W_1o / np.sqrt(384.0)).astype(ndt),
        "l1e": np.ascontiguousarray(W_1e / np.sqrt(128.0)).astype(ndt),
    }
    feat = np.asarray(node_feat, dtype=np.float32).reshape(N_CORES, NS, 640)
    in_maps = []
    for i in range(N_CORES):
        blk = feat[i]
        xT = np.zeros((640, NSH), ndt)
        xT[:256, :NS] = blk[:, :256].T.astype(ndt)
        vv = blk[:, 256:].reshape(NS, 128, 3)
        xT[256:, :NS] = vv.transpose(2, 1, 0).reshape(384, NS).astype(ndt)
        in_maps.append({"xT": xT, **weights})
    return in_maps


def _gather(results):
    out = np.empty((N_NODES, 1024), np.float32)
    for i in range(N_CORES):
        oT = np.asarray(results[i]["outT"]).astype(np.float32,
                                                   copy=False)[:, :NS]
        blk = out[i * NS:(i + 1) * NS]
        blk[:, :256] = oT[:256].T
        blk[:, 256:640] = oT[256:640].reshape(3, 128, NS).transpose(2, 1, 0) \
            .reshape(NS, 384)
        blk[:, 640:] = oT[640:].reshape(3, 128, NS).transpose(2, 1, 0) \
            .reshape(NS, 384)
    return out


def kernel(node_feat, w_00_0, w_01_1, w_10_1, w_11_0, w_11_1,
           W_0e, W_1o, W_1e, _trace=False):
    if VARIANT not in _CACHE:
        _CACHE[VARIANT] = _build_program(VARIANT)
    nc = _CACHE[VARIANT]
    in_maps = _prep_inputs(node_feat, w_00_0, w_01_1, w_10_1, w_11_0,
                           w_11_1, W_0e, W_1o, W_1e,
                           b16=(VARIANT == "b16"))
    res = run_bass_kernel_spmd(nc, in_maps, core_ids=list(range(N_CORES)),
                               trace=_trace)
    out = _gather(res.results)
    if _trace:
        return out, res
    return out



# revision 2
# speedup vs baseline: 1.0947x; 1.0947x over previous
"""Trainium2 Bass kernel for nn_NodePreTrans (e3nn tensor product + linear).

Data-parallel over nodes: 50000 rows sharded 8 ways (6250/core, padded to
6272).  Channel-major on-device layout: all matmuls are (weights stationary)
[K,128] x [K,Z] with Z up to 512 nodes in the moving/free dimension.
"""

import sys

sys.path.insert(0, "/opt/trn_rl_repo")

import numpy as np

import concourse.bacc as bacc
import concourse.bass as bass
import concourse.mybir as mybir
import concourse.tile as tile
from concourse.bass_utils import run_bass_kernel_spmd

N_NODES = 50000
N_CORES = 8
NS = N_NODES // N_CORES          # 6250 real nodes per core
NSH = 6272                       # padded (12*512 + 128)
MUL_S = 256
MUL_V = 128

C_000 = 1.0 / np.sqrt(256.0)
C_011 = 1.0 / np.sqrt(128.0)
C_101 = 1.0 / np.sqrt(256.0)
C_110 = 1.0 / np.sqrt(384.0)
C_111 = 1.0 / 16.0

F32 = mybir.dt.float32
F32R = mybir.dt.float32r
BF16 = mybir.dt.bfloat16

_CACHE = {}

VARIANT = "b16"


def _build_program(variant="full"):
    nc = bacc.Bacc("TRN2", target_bir_lowering=False, debug=False,
                   num_devices=N_CORES)

    if variant == "b16":
        MDT = BF16
    elif variant in ("dma", "mm"):
        MDT = F32
    else:
        MDT = F32R
    ODT = BF16 if variant == "b16" else F32
    xT_d = nc.dram_tensor("xT", [640, NSH], MDT, kind="ExternalInput").ap()
    wt000_d = nc.dram_tensor("wt000", [256, 256], MDT, kind="ExternalInput").ap()
    wt011_d = nc.dram_tensor("wt011", [128, 256], MDT, kind="ExternalInput").ap()
    wt101_d = nc.dram_tensor("wt101", [256, 128], MDT, kind="ExternalInput").ap()
    wt110_d = nc.dram_tensor("wt110", [128, 128], MDT, kind="ExternalInput").ap()
    wt111_d = nc.dram_tensor("wt111", [128, 128], MDT, kind="ExternalInput").ap()
    l0e_d = nc.dram_tensor("l0e", [384, 256], MDT, kind="ExternalInput").ap()
    l1o_d = nc.dram_tensor("l1o", [384, 128], MDT, kind="ExternalInput").ap()
    l1e_d = nc.dram_tensor("l1e", [128, 128], MDT, kind="ExternalInput").ap()
    outT_d = nc.dram_tensor("outT", [1024, NSH], ODT, kind="ExternalOutput").ap()

    with tile.TileContext(nc) as tc:
        _emit(tc, nc, xT_d, wt000_d, wt011_d, wt101_d, wt110_d, wt111_d,
              l0e_d, l1o_d, l1e_d, outT_d, variant=variant, mdt=MDT)

    nc.compile()
    return nc


def _emit(tc, nc, xT_d, wt000_d, wt011_d, wt101_d, wt110_d, wt111_d,
          l0e_d, l1o_d, l1e_d, outT_d, variant="full", mdt=F32R):
    if variant == "b16":
        TW = 1024                      # bf16 moving + bf16 PSUM bank limit
        PDT = EDT = BF16
        zblocks = [(i * 1024, 1024) for i in range(6)] + [(6144, 128)]
    else:
        TW = 512                       # f32 PSUM bank limit
        PDT = EDT = F32
        zblocks = [(i * 512, 512) for i in range(12)] + [(6144, 128)]
    with (
        tc.tile_pool(name="wpool", bufs=1) as wpool,
        tc.tile_pool(name="xin", bufs=3) as xin,
        tc.tile_pool(name="gat", bufs=2) as gat,
        tc.tile_pool(name="tmp", bufs=4) as tmp,
        tc.tile_pool(name="oev", bufs=2) as oev,
        tc.tile_pool(name="ps1", bufs=1, space="PSUM") as ps1,
        tc.tile_pool(name="ps2", bufs=1, space="PSUM") as ps2,
    ):
        # ---- resident weights -------------------------------------------
        # trigger weight loads from ACT so they don't queue ahead of the
        # x-tile loads on Sync's DGE ring; order them by first use.
        def wtile(name, dram_ap, rows, cols):
            t = wpool.tile([128, cols], mdt, name=name)
            nc.scalar.dma_start(t[:, :], dram_ap[rows:rows + 128, :])
            return t

        w111 = wtile("w111", wt111_d, 0, 128)
        w110 = wtile("w110", wt110_d, 0, 128)
        w011 = wtile("w011", wt011_d, 0, 256)
        w000 = [wtile(f"w000_{k}", wt000_d, 128 * k, 256) for k in range(2)]
        w101 = [wtile(f"w101_{k}", wt101_d, 128 * k, 128) for k in range(2)]
        L1e = wtile("l1e", l1e_d, 0, 128)
        L1o = [wtile(f"l1o_{k}", l1o_d, 128 * k, 128) for k in range(3)]
        L0e = [wtile(f"l0e_{k}", l0e_d, 128 * k, 256) for k in range(3)]

        for bi, (z0, Z) in enumerate(zblocks):
            # ---- load x tiles (channel-major); v first (path 5 needs it)
            def load(t, row0, Z=Z, z0=z0):
                nc.sync.dma_start(t[:, :Z], xT_d[row0:row0 + 128,
                                                 z0:z0 + Z])

            v = []
            for j in range(3):
                t = xin.tile([128, TW], mdt, name=f"v{j}")
                load(t, 256 + 128 * j)
                v.append(t)
            s = []
            for m in range(2):
                t = xin.tile([128, TW], mdt, name=f"s{m}")
                load(t, 128 * m)
                s.append(t)

            def ps_tile():
                return ps1.tile([128, TW], PDT, name="s1r", bufs=5)

            def mmr(out, lhsT, rhs, start, stop):
                nc.tensor.matmul(out, lhsT, rhs, start=start, stop=stop)

            if variant == "dma":
                for i, t in enumerate(s + v):
                    nc.sync.dma_start(outT_d[128 * i:128 * (i + 1),
                                             z0:z0 + Z], t[:, :Z])
                continue

            if variant == "mm":
                idx = 0
                for (w, rr) in [(w000[0], s[0]), (w000[1], s[1]),
                                (w011, v[0]), (w011, v[1]), (w011, v[2]),
                                (w101[0], s[0]), (w101[1], s[1]),
                                (w110, v[0]), (w110, v[1]), (w110, v[2]),
                                (w111, v[0]), (w111, v[1]), (w111, v[2])]:
                    a = ps_tile()
                    nc.tensor.matmul(a[:, :Z], w[:, :128], rr[:, :Z],
                                     start=True, stop=True)
                    ev = oev.tile([128, TW], EDT, name=f"mmev{idx % 4}")
                    nc.scalar.copy(ev[:, :Z], a[:, :Z])
                    nc.sync.dma_start(outT_d[128 * (idx % 8):
                                             128 * (idx % 8) + 128,
                                             z0:z0 + Z], ev[:, :Z])
                    idx += 1
                continue

            # ---- path 5: p5_k = v_i*E_j - v_j*E_i, (i,j)=(k+1,k+2)%3 ---
            # muls on DVE (read E from PSUM); final subs on GpSimd (SBUF-only)
            E = []
            for j in range(3):
                e = ps_tile()
                mmr(e[:, :Z], w111[:, :], v[j][:, :Z], start=True, stop=True)
                E.append(e)
            p5 = []
            for k in range(3):
                i, j = (k + 1) % 3, (k + 2) % 3
                ta = tmp.tile([128, TW], mdt, name="t5a")
                tb = tmp.tile([128, TW], mdt, name="t5b")
                nc.vector.tensor_mul(ta[:, :Z], v[i][:, :Z], E[j][:, :Z])
                nc.vector.tensor_mul(tb[:, :Z], v[j][:, :Z], E[i][:, :Z])
                p = gat.tile([128, TW], mdt, name=f"p5_{k}")
                nc.gpsimd.tensor_sub(p[:, :Z], ta[:, :Z], tb[:, :Z])
                p5.append(p)

            # ---- path 1: p1 = s * (w00.T @ s) --------------------------
            p1 = []
            for m in range(2):
                a = ps_tile()
                mmr(a[:, :Z], w000[0][:, 128 * m:128 * (m + 1)],
                    s[0][:, :Z], start=True, stop=False)
                mmr(a[:, :Z], w000[1][:, 128 * m:128 * (m + 1)],
                    s[1][:, :Z], start=False, stop=True)
                p = gat.tile([128, TW], mdt, name=f"p1_{m}")
                nc.vector.tensor_mul(p[:, :Z], s[m][:, :Z], a[:, :Z])
                p1.append(p)

            # ---- path 2: p2_j = s * (w01.T @ v_j) ----------------------
            p2 = []
            for j in range(3):
                pj = []
                for m in range(2):
                    b = ps_tile()
                    mmr(b[:, :Z], w011[:, 128 * m:128 * (m + 1)],
                        v[j][:, :Z], start=True, stop=True)
                    p = gat.tile([128, TW], mdt, name=f"p2_{j}_{m}")
                    nc.vector.tensor_mul(p[:, :Z], s[m][:, :Z], b[:, :Z])
                    pj.append(p)
                p2.append(pj)

            # ---- path 3: p3_j = v_j * (w10.T @ s) ----------------------
            c = ps_tile()
            mmr(c[:, :Z], w101[0][:, :], s[0][:, :Z], start=True, stop=False)
            mmr(c[:, :Z], w101[1][:, :], s[1][:, :Z], start=False, stop=True)
            p3 = []
            for j in range(3):
                p = gat.tile([128, TW], mdt, name=f"p3_{j}")
                nc.vector.tensor_mul(p[:, :Z], v[j][:, :Z], c[:, :Z])
                p3.append(p)

            # ---- path 4: p4 = sum_j v_j * (w110.T @ v_j) ---------------
            # muls on DVE (read PSUM); accumulate adds on GpSimd (SBUF-only)
            p4 = gat.tile([128, TW], mdt, name="p4")
            for j in range(3):
                d = ps_tile()
                mmr(d[:, :Z], w110[:, :], v[j][:, :Z], start=True, stop=True)
                if j == 0:
                    nc.vector.tensor_mul(p4[:, :Z], v[0][:, :Z], d[:, :Z])
                else:
                    t4 = tmp.tile([128, TW], mdt, name="t4")
                    nc.vector.tensor_mul(t4[:, :Z], v[j][:, :Z], d[:, :Z])
                    nc.gpsimd.tensor_add(p4[:, :Z], p4[:, :Z], t4[:, :Z])

            if variant == "gat":
                outs8 = [p1[0], p1[1], p2[0][0], p2[0][1], p3[0], p4,
                         p5[0], p5[1]]
                for i, t in enumerate(outs8):
                    nc.sync.dma_start(outT_d[128 * i:128 * (i + 1),
                                             z0:z0 + Z], t[:, :Z])
                continue

            # ---- stage 2 linears + evacuate + store --------------------
            # last two blocks: split store triggers across Sync+ACT so the
            # end-of-kernel triggers don't serialize (ACT is idle by then)
            tail = bi >= len(zblocks) - 2
            oidx = [0]

            def emit_out(name, row0, chunks):
                o = ps2.tile([128, TW], PDT, name="s2o", bufs=3)
                n = len(chunks)
                for ci, (lw, rhs) in enumerate(chunks):
                    mmr(o[:, :Z], lw, rhs[:, :Z],
                        start=(ci == 0), stop=(ci == n - 1))
                ev = oev.tile([128, TW], EDT, name=name)
                nc.scalar.copy(ev[:, :Z], o[:, :Z])
                eng = nc.scalar if (tail and oidx[0] % 2) else nc.sync
                oidx[0] += 1
                eng.dma_start(outT_d[row0:row0 + 128, z0:z0 + Z],
                              ev[:, :Z])

            for j in range(3):
                emit_out(f"o1e_{j}", 640 + 128 * j, [(L1e[:, :], p5[j])])
            for j in range(3):
                tp1o = [p2[j][0], p2[j][1], p3[j]]
                emit_out(f"o1o_{j}", 256 + 128 * j,
                         [(L1o[ci][:, :], tp1o[ci]) for ci in range(3)])
            tp0e = [p1[0], p1[1], p4]
            for m in range(2):
                emit_out(f"o0e_{m}", 128 * m,
                         [(L0e[ci][:, 128 * m:128 * (m + 1)], tp0e[ci])
                          for ci in range(3)])


def _prep_inputs(node_feat, w_00_0, w_01_1, w_10_1, w_11_0, w_11_1,
                 W_0e, W_1o, W_1e, b16=False):
    ndt = np.float32
    if b16:
        import ml_dtypes
        ndt = ml_dtypes.bfloat16
    weights = {
        "wt000": np.ascontiguousarray((C_000 * w_00_0).T).astype(ndt),
        "wt011": np.ascontiguousarray((C_011 * w_01_1).T).astype(ndt),
        "wt101": np.ascontiguousarray((C_101 * w_10_1).T).astype(ndt),
        "wt110": np.ascontiguousarray((C_110 * w_11_0).T).astype(ndt),
        "wt111": np.ascontiguousarray((C_111 * w_11_1).T).astype(ndt),
        "l0e": np.ascontiguousarray(W_0e / np.sqrt(384.0)).astype(ndt),
        "l1o": np.ascontiguousarray(W_1o / np.sqrt(384.0)).astype(ndt),
        "l1e": np.ascontiguousarray(W_1e / np.sqrt(128.0)).astype(ndt),
    }
    feat = np.asarray(node_feat, dtype=np.float32).reshape(N_CORES, NS, 640)
    in_maps = []
    for i in range(N_CORES):
        blk = feat[i]
        xT = np.zeros((640, NSH), ndt)
        xT[:256, :NS] = blk[:, :256].T.astype(ndt)
        vv = blk[:, 256:].reshape(NS, 128, 3)
        xT[256:, :NS] = vv.transpose(2, 1, 0).reshape(384, NS).astype(ndt)
        in_maps.append({"xT": xT, **weights})
    return in_maps


def _gather(results):
    out = np.empty((N_NODES, 1024), np.float32)
    for i in range(N_CORES):
        oT = np.asarray(results[i]["outT"]).astype(np.float32,
                                                   copy=False)[:, :NS]
        blk = out[i * NS:(i + 1) * NS]
        blk[:, :256] = oT[:256].T
        blk[:, 256:640] = oT[256:640].reshape(3, 128, NS).transpose(2, 1, 0) \
            .reshape(NS, 384)
        blk[:, 640:] = oT[640:].reshape(3, 128, NS).transpose(2, 1, 0) \
            .reshape(NS, 384)
    return out


def kernel(node_feat, w_00_0, w_01_1, w_10_1, w_11_0, w_11_1,
           W_0e, W_1o, W_1e, _trace=False):
    if VARIANT not in _CACHE:
        _CACHE[VARIANT] = _build_program(VARIANT)
    nc = _CACHE[VARIANT]
    in_maps = _prep_inputs(node_feat, w_00_0, w_01_1, w_10_1, w_11_0,
                           w_11_1, W_0e, W_1o, W_1e,
                           b16=(VARIANT == "b16"))
    res = run_bass_kernel_spmd(nc, in_maps, core_ids=list(range(N_CORES)),
                               trace=_trace)
    out = _gather(res.results)
    if _trace:
        return out, res
    return out



# revision 3
# speedup vs baseline: 1.1360x; 1.0377x over previous
"""Trainium2 Bass kernel for nn_NodePreTrans (e3nn tensor product + linear).

v3 design "T": all-bf16 datapath (x, weights, p-tiles, stores), f32 PSUM.

Engine budget per 512-col z-block (measured rates):
  PE  : 39 MMs (36 + 3 extra: p5 sub folded into o1e via negated L1e)
  DVE : 3 mixed muls (sbuf x psum, 1x) + A/B/p2g4/p4-adds at bf16 2x
  ACT : Ewc copy, g4 copy, 2 wide evacs, store trigger (all 2048-wide)
  GpS : p3 only (bf16, slow but fits the shared-port budget with DVE-2x)
Port rules: DVE-2x (two SBUF reads) and GpSimd exclude each other; DVE
mixed/psum ops don't touch the shared pair.
PSUM: one pool of [128, 2048] grabs (4 banks) x bufs=2 = 8 banks.
outT row order: [o0e(2) | o1e(3) | o1o(3)] so both stage-2 evacs are
contiguous 2048-wide copies.
"""

import sys

sys.path.insert(0, "/opt/trn_rl_repo")

import numpy as np

import concourse.bacc as bacc
import concourse.bass as bass
import concourse.mybir as mybir
import concourse.tile as tile
from concourse.bass_utils import run_bass_kernel_spmd

N_NODES = 50000
N_CORES = 8
NS = N_NODES // N_CORES
NSH = 6272                       # 12*512 + 128
TW = 512

C_000 = 1.0 / np.sqrt(256.0)
C_011 = 1.0 / np.sqrt(128.0)
C_101 = 1.0 / np.sqrt(256.0)
C_110 = 1.0 / np.sqrt(384.0)
C_111 = 1.0 / 16.0

F32 = mybir.dt.float32
BF16 = mybir.dt.bfloat16

_CACHE = {}


def _ap3(ap2, n, w):
    """View a 2D contiguous AP [P, n*w] as [P, n, w]."""
    (ps, pn), (s, c) = ap2.ap[0], ap2.ap[1]
    assert s == 1 and c == n * w, (ap2.ap, n, w)
    return bass.AP(ap2.tensor, ap2.offset, [(ps, pn), (w, n), (1, w)])


def _bcast(ap2, n):
    """Broadcast a 2D AP [P, w] to [P, n, w] with a stride-0 dim."""
    (ps, pn), (s, c) = ap2.ap[0], ap2.ap[1]
    return bass.AP(ap2.tensor, ap2.offset, [(ps, pn), (0, n), (s, c)])


def _dram3(dram_ap, row0, n, z0, Z, nsh=NSH):
    """[128, n, Z] view over dram [rows, NSH]; row = row0 + 128*chunk + p."""
    base = dram_ap[row0:row0 + 128, z0:z0 + Z]
    return bass.AP(base.tensor, base.offset,
                   [(nsh, 128), (128 * nsh, n), (1, Z)])


def _build_program():
    nc = bacc.Bacc("TRN2", target_bir_lowering=False, debug=False,
                   num_devices=N_CORES)

    xT_d = nc.dram_tensor("xT", [640, NSH], BF16, kind="ExternalInput").ap()
    wt000_d = nc.dram_tensor("wt000", [256, 256], BF16, kind="ExternalInput").ap()
    wt011_d = nc.dram_tensor("wt011", [128, 256], BF16, kind="ExternalInput").ap()
    wt101_d = nc.dram_tensor("wt101", [256, 128], BF16, kind="ExternalInput").ap()
    wt110_d = nc.dram_tensor("wt110", [128, 128], BF16, kind="ExternalInput").ap()
    wt111_d = nc.dram_tensor("wt111", [128, 128], BF16, kind="ExternalInput").ap()
    l0e_d = nc.dram_tensor("l0e", [384, 256], BF16, kind="ExternalInput").ap()
    l1o_d = nc.dram_tensor("l1o", [384, 128], BF16, kind="ExternalInput").ap()
    l1e_d = nc.dram_tensor("l1e", [128, 256], BF16, kind="ExternalInput").ap()
    outT_d = nc.dram_tensor("outT", [1024, NSH], BF16, kind="ExternalOutput").ap()

    with tile.TileContext(nc) as tc:
        _emit(tc, nc, xT_d, wt000_d, wt011_d, wt101_d, wt110_d, wt111_d,
              l0e_d, l1o_d, l1e_d, outT_d)

    nc.compile()
    return nc


def _emit(tc, nc, xT_d, wt000_d, wt011_d, wt101_d, wt110_d, wt111_d,
          l0e_d, l1o_d, l1e_d, outT_d):
    zblocks = [(i * TW, TW) for i in range(12)] + [(6144, 128)]
    with (
        tc.tile_pool(name="wpool", bufs=1) as wpool,
        tc.tile_pool(name="xin", bufs=4) as xin,
        tc.tile_pool(name="mid", bufs=3) as mid,
        tc.tile_pool(name="pt", bufs=3) as pt,
        tc.tile_pool(name="ost", bufs=3) as ost,
        tc.tile_pool(name="ps1", bufs=1, space="PSUM") as ps1,
    ):
        # warmup MMs keep PE busy (HAM un-throttle) while first loads land
        junk = wpool.tile([128, 256], BF16, name="junk")
        nc.gpsimd.memset(junk[:, :], 0.125)
        wps = ps1.tile([128, 2 * TW], F32, name="s1", bufs=4)
        for _ in range(30):
            nc.tensor.matmul(wps[:, :128], junk[:, 0:128], junk[:, 128:256],
                             start=True, stop=True)

        def wtile(name, dram_ap, rows, cols):
            t = wpool.tile([128, cols], BF16, name=name)
            nc.scalar.dma_start(t[:, :], dram_ap[rows:rows + 128, :])
            return t

        w111 = wtile("w111", wt111_d, 0, 128)
        w101 = [wtile(f"w101_{k}", wt101_d, 128 * k, 128) for k in range(2)]
        w000 = [wtile(f"w000_{k}", wt000_d, 128 * k, 256) for k in range(2)]
        w110 = wtile("w110", wt110_d, 0, 128)
        w011 = wtile("w011", wt011_d, 0, 256)
        L0e = [wtile(f"l0e_{k}", l0e_d, 128 * k, 256) for k in range(3)]
        # l1e dram holds [L1e | -L1e] as 256 cols
        L1e2 = wtile("l1e2", l1e_d, 0, 256)
        L1o = [wtile(f"l1o_{k}", l1o_d, 128 * k, 128) for k in range(3)]

        def mm(out, lhsT, rhs, start=True, stop=True):
            nc.tensor.matmul(out, lhsT, rhs, start=start, stop=stop)

        def stage1(bi):
            z0, Z = zblocks[bi]
            x_w = xin.tile([128, 5 * TW], BF16, name="x_w")
            nc.sync.dma_start(_ap3(x_w[:, :5 * Z], 5, Z),
                              _dram3(xT_d, 0, 5, z0, Z))

            def sx(i, n=1):
                return x_w[:, i * Z:(i + n) * Z]

            def vx(i, n=1):
                return x_w[:, (2 + i) * Z:(2 + i + n) * Z]

            def grab():
                return ps1.tile([128, 2 * TW], F32, name="s1", bufs=4)

            # ---- h1 = [E1|E2], h2 = [E0|c]  (E rotated) -------------------
            h1 = grab()
            mm(h1[:, 0:Z], w111[:, :], vx(1))
            mm(h1[:, Z:2 * Z], w111[:, :], vx(2))
            h2 = grab()
            mm(h2[:, 0:Z], w111[:, :], vx(0))
            mm(h2[:, Z:2 * Z], w101[0][:, :], sx(0), start=True, stop=False)
            mm(h2[:, Z:2 * Z], w101[1][:, :], sx(1), start=False, stop=True)
            Ewc = mid.tile([128, 4 * TW], BF16, name="Ewc")
            nc.scalar.copy(Ewc[:, 0:2 * Z], h1[:, :2 * Z])
            nc.scalar.copy(Ewc[:, 2 * Z:4 * Z], h2[:, :2 * Z])

            def ew(i, n=1):
                return Ewc[:, i * Z:(i + n) * Z]

            cw = Ewc[:, 3 * Z:4 * Z]

            # ---- h3 = [a0|a1] -> p1 ---------------------------------------
            h3 = grab()
            for m in range(2):
                mm(h3[:, m * Z:(m + 1) * Z],
                   w000[0][:, 128 * m:128 * (m + 1)], sx(0),
                   start=True, stop=False)
                mm(h3[:, m * Z:(m + 1) * Z],
                   w000[1][:, 128 * m:128 * (m + 1)], sx(1),
                   start=False, stop=True)
            p1 = pt.tile([128, 2 * TW], BF16, name="p1")
            nc.vector.tensor_mul(p1[:, :2 * Z], x_w[:, :2 * Z], h3[:, :2 * Z])

            # ---- h4 = [d0|d1], h5 = [d2|-] -> p4 (early for k1) -----------
            h4 = grab()
            mm(h4[:, 0:Z], w110[:, :], vx(0))
            mm(h4[:, Z:2 * Z], w110[:, :], vx(1))
            p4t = pt.tile([128, 3 * TW], BF16, name="p4t")
            nc.vector.tensor_mul(p4t[:, :2 * Z], vx(0, 2), h4[:, :2 * Z])
            h5 = grab()
            mm(h5[:, 0:Z], w110[:, :], vx(2))
            p4t2 = pt.tile([128, TW], BF16, name="p4t2")
            nc.vector.tensor_mul(p4t2[:, :Z], vx(2), h5[:, 0:Z])
            p4 = pt.tile([128, TW], BF16, name="p4")
            nc.gpsimd.tensor_add(p4[:, :Z], p4t[:, :Z], p4t[:, Z:2 * Z])
            nc.gpsimd.tensor_add(p4[:, :Z], p4[:, :Z], p4t2[:, :Z])

            # ---- GpS: p3 = v * bcast(c) -----------------------------------
            p3 = pt.tile([128, 3 * TW], BF16, name="p3")
            nc.gpsimd.tensor_mul(_ap3(p3[:, :3 * Z], 3, Z),
                                 _ap3(x_w[:, 2 * Z:5 * Z], 3, Z),
                                 _bcast(cw[:, :Z], 3))

            # ---- h6..h8 = [b_j0|b_j1] -> p2_j -----------------------------
            p2 = pt.tile([128, 6 * TW], BF16, name="p2")
            for j in range(3):
                h = grab()
                mm(h[:, 0:Z], w011[:, 0:128], vx(j))
                mm(h[:, Z:2 * Z], w011[:, 128:256], vx(j))
                nc.vector.tensor_mul(p2[:, j * 2 * Z:(j + 1) * 2 * Z],
                                     x_w[:, :2 * Z], h[:, :2 * Z])

            # ---- DVE 2x: A/B products (late; consumed next step) ----------
            A = mid.tile([128, 3 * TW], BF16, name="A")
            nc.vector.tensor_mul(A[:, 0:2 * Z], vx(1, 2), ew(1, 2))
            nc.vector.tensor_mul(A[:, 2 * Z:3 * Z], vx(0), ew(0))
            B = mid.tile([128, 3 * TW], BF16, name="B")
            nc.vector.tensor_mul(B[:, 0:Z], vx(2), ew(0))
            nc.vector.tensor_mul(B[:, Z:3 * Z], vx(0, 2), ew(1, 2))
            return dict(z0=z0, Z=Z, p1=p1, p4=p4, p2=p2, p3=p3, A=A, B=B)

        def stage2(st, split_store=False):
            z0, Z = st["z0"], st["Z"]
            p1, p4, p2, p3, A, B = (st["p1"], st["p4"], st["p2"], st["p3"],
                                    st["A"], st["B"])
            stor = ost.tile([128, 8 * TW], BF16, name="stor")

            def grab():
                return ps1.tile([128, 2 * TW], F32, name="s1", bufs=4)

            k1 = grab()
            for m in range(2):
                o = k1[:, m * Z:(m + 1) * Z]
                mm(o, L0e[0][:, 128 * m:128 * (m + 1)], p1[:, :Z],
                   start=True, stop=False)
                mm(o, L0e[1][:, 128 * m:128 * (m + 1)], p1[:, Z:2 * Z],
                   start=False, stop=False)
                mm(o, L0e[2][:, 128 * m:128 * (m + 1)], p4[:, :Z],
                   start=False, stop=True)
            nc.scalar.copy(stor[:, 0:2 * Z], k1[:, :2 * Z])
            if split_store:
                nc.sync.dma_start(_dram3(outT_d, 0, 2, z0, Z),
                                  _ap3(stor[:, 0:2 * Z], 2, Z))
            k2 = grab()
            for j in range(2):
                o = k2[:, j * Z:(j + 1) * Z]
                mm(o, L1e2[:, 0:128], A[:, j * Z:(j + 1) * Z],
                   start=True, stop=False)
                mm(o, L1e2[:, 128:256], B[:, j * Z:(j + 1) * Z],
                   start=False, stop=True)
            nc.scalar.copy(stor[:, 2 * Z:4 * Z], k2[:, :2 * Z])
            if split_store:
                nc.sync.dma_start(_dram3(outT_d, 256, 2, z0, Z),
                                  _ap3(stor[:, 2 * Z:4 * Z], 2, Z))

            def o1o(o, j):
                mm(o, L1o[0][:, :], p2[:, j * 2 * Z:j * 2 * Z + Z],
                   start=True, stop=False)
                mm(o, L1o[1][:, :], p2[:, j * 2 * Z + Z:(j + 1) * 2 * Z],
                   start=False, stop=False)
                mm(o, L1o[2][:, :], p3[:, j * Z:(j + 1) * Z],
                   start=False, stop=True)

            k3 = grab()
            mm(k3[:, 0:Z], L1e2[:, 0:128], A[:, 2 * Z:3 * Z],
               start=True, stop=False)
            mm(k3[:, 0:Z], L1e2[:, 128:256], B[:, 2 * Z:3 * Z],
               start=False, stop=True)
            o1o(k3[:, Z:2 * Z], 0)
            nc.scalar.copy(stor[:, 4 * Z:6 * Z], k3[:, :2 * Z])
            if split_store:
                nc.sync.dma_start(_dram3(outT_d, 512, 2, z0, Z),
                                  _ap3(stor[:, 4 * Z:6 * Z], 2, Z))
            k4 = grab()
            o1o(k4[:, 0:Z], 1)
            o1o(k4[:, Z:2 * Z], 2)
            nc.scalar.copy(stor[:, 6 * Z:8 * Z], k4[:, :2 * Z])
            if split_store:
                nc.sync.dma_start(_dram3(outT_d, 768, 2, z0, Z),
                                  _ap3(stor[:, 6 * Z:8 * Z], 2, Z))
                return None
            return dict(z0=z0, Z=Z, stor=stor)

        def store(st):
            z0, Z, stor = st["z0"], st["Z"], st["stor"]
            nc.sync.dma_start(_dram3(outT_d, 0, 8, z0, Z),
                              _ap3(stor[:, :8 * Z], 8, Z))

        # software pipeline: [load+stage1(i), store(i-2), stage2(i-1)]
        pend1 = None
        pend2 = None
        for bi in range(len(zblocks)):
            st = stage1(bi)
            if pend2 is not None:
                store(pend2)
            if pend1 is not None:
                pend2 = stage2(pend1)
            pend1 = st
        store(pend2)
        stage2(pend1, split_store=True)


def _prep_inputs(node_feat, w_00_0, w_01_1, w_10_1, w_11_0, w_11_1,
                 W_0e, W_1o, W_1e):
    import ml_dtypes
    ndt = ml_dtypes.bfloat16
    l1e = W_1e / np.sqrt(128.0)
    l1e2 = np.concatenate([l1e, -l1e], axis=1)        # [128, 256]
    weights = {
        "wt000": np.ascontiguousarray((C_000 * w_00_0).T).astype(ndt),
        "wt011": np.ascontiguousarray((C_011 * w_01_1).T).astype(ndt),
        "wt101": np.ascontiguousarray((C_101 * w_10_1).T).astype(ndt),
        "wt110": np.ascontiguousarray((C_110 * w_11_0).T).astype(ndt),
        "wt111": np.ascontiguousarray((C_111 * w_11_1).T).astype(ndt),
        "l0e": np.ascontiguousarray(W_0e / np.sqrt(384.0)).astype(ndt),
        "l1o": np.ascontiguousarray(W_1o / np.sqrt(384.0)).astype(ndt),
        "l1e": np.ascontiguousarray(l1e2).astype(ndt),
    }
    feat = np.asarray(node_feat, dtype=np.float32).reshape(N_CORES, NS, 640)
    in_maps = []
    for i in range(N_CORES):
        blk = feat[i]
        xT = np.zeros((640, NSH), ndt)
        xT[:256, :NS] = blk[:, :256].T.astype(ndt)
        vv = blk[:, 256:].reshape(NS, 128, 3)
        xT[256:, :NS] = vv.transpose(2, 1, 0).reshape(384, NS).astype(ndt)
        in_maps.append({"xT": xT, **weights})
    return in_maps


def _gather(results):
    # outT rows: [o0e(0:256) | o1e(256:640) | o1o(640:1024)]
    out = np.empty((N_NODES, 1024), np.float32)
    for i in range(N_CORES):
        oT = np.asarray(results[i]["outT"]).astype(np.float32,
                                                   copy=False)[:, :NS]
        blk = out[i * NS:(i + 1) * NS]
        blk[:, :256] = oT[:256].T
        blk[:, 640:] = oT[256:640].reshape(3, 128, NS).transpose(2, 1, 0) \
            .reshape(NS, 384)
        blk[:, 256:640] = oT[640:].reshape(3, 128, NS).transpose(2, 1, 0) \
            .reshape(NS, 384)
    return out


def kernel(node_feat, w_00_0, w_01_1, w_10_1, w_11_0, w_11_1,
           W_0e, W_1o, W_1e, _trace=False):
    if "v3" not in _CACHE:
        _CACHE["v3"] = _build_program()
    nc = _CACHE["v3"]
    in_maps = _prep_inputs(node_feat, w_00_0, w_01_1, w_10_1, w_11_0,
                           w_11_1, W_0e, W_1o, W_1e)
    res = run_bass_kernel_spmd(nc, in_maps, core_ids=list(range(N_CORES)),
                               trace=_trace)
    out = _gather(res.results)
    if _trace:
        return out, res
    return out
